# revision 4
# baseline (speedup 1.0000x reference)
"""CGMM layer-0 forward on 8 Trainium2 NeuronCores.

Math: per-node likelihood depends only on the node's discrete label
x[n] in [0, 32), so
    lik_node[n, :] = L[x[n], :]         with L a (32, 16) table
    lik_graph[s,:] = sum_m count[s, m] * L[m, :]
where count[s, m] = #{nodes of graph s with label m}.

Sharding: graphs are pre-partitioned across the 8 cores (625 graphs
each, zero cross-core traffic).  Host packs each graph's labels into a
fixed-stride padded row (pad value 64 -> matches no label).  Device:
  - builds per-label one-hots with DVE tensor_scalar is_equal (bf16, 4x)
  - reduces over slots with a contiguous-halves TT-add tree (bf16, 2x)
  - computes L on-device from B, Pi (softmaxes / log via ACT+PE)
  - count @ L via PE transpose + matmul, negate on psum->sbuf copy
"""

import math

import numpy as np

N_NODES = 500_000
N_GRAPHS = 5_000
C = 16
M = 32
G = 16
N_CORES = 8
GPC = N_GRAPHS // N_CORES  # graphs per core = 625
J = 5                      # partition blocks of 128 graphs
GPAD = 128 * J             # padded graphs per core = 640
PAD_LABEL = 64.0


def _build_nc(T):
    import concourse.bass as bass
    import concourse.bacc as bacc
    import concourse.tile as tile
    import concourse.mybir as mybir
    from concourse.masks import make_identity

    fp32 = mybir.dt.float32
    bf16 = mybir.dt.bfloat16
    Alu = mybir.AluOpType
    Act = mybir.ActivationFunctionType

    nc = bacc.Bacc("TRN2", target_bir_lowering=False, debug=False)

    xp_d = nc.dram_tensor("xp", [GPAD, T], bf16, kind="ExternalInput").ap()
    b_d = nc.dram_tensor("B", [C, M * G], fp32, kind="ExternalInput").ap()
    pi_d = nc.dram_tensor("Pi", [C, G], fp32, kind="ExternalInput").ap()
    out_d = nc.dram_tensor("out", [GPAD, G], fp32, kind="ExternalOutput").ap()

    with tile.TileContext(nc) as tc:
        with (
            tc.tile_pool(name="main", bufs=1) as main,
            tc.tile_pool(name="tree", bufs=2) as tree,
            tc.tile_pool(name="psum", bufs=2, space="PSUM") as pp,
            tc.tile_pool(name="psum1", bufs=1, space="PSUM") as pp1,
        ):
            # ---- constants ----
            ones = main.tile([C, G], fp32)
            nc.vector.memset(ones, 1.0)
            identB = main.tile([128, 128], bf16)
            make_identity(nc, identB)

            # ---- loads ----
            XP = main.tile([128, J, T], bf16)
            nc.sync.dma_start(out=XP, in_=xp_d.rearrange("(j p) t -> p j t", p=128))
            Bt = main.tile([C, M * G], fp32)
            nc.sync.dma_start(out=Bt, in_=b_d)
            Pit = main.tile([C, G], fp32)
            nc.sync.dma_start(out=Pit, in_=pi_d)

            # ---- L table: L[m,g] = sum_c post*log(num) ----
            # Pi softmax over c (partition axis -> via PE ones-matmuls)
            expPi = main.tile([C, G], fp32)
            nc.scalar.activation(expPi, Pit, Act.Exp)
            spi = pp1.tile([1, G], fp32)
            nc.tensor.matmul(spi, ones[:, 0:1], expPi, start=True, stop=True)
            rspi = main.tile([1, G], fp32)
            nc.vector.reciprocal(rspi, spi)
            rspiB = pp1.tile([C, G], fp32)
            nc.tensor.matmul(rspiB, ones[0:1, :], rspi, start=True, stop=True)
            smPi = main.tile([C, G], fp32)
            nc.vector.tensor_mul(smPi, expPi, rspiB)

            # B softmax over m (free axis)
            expB = main.tile([C, M * G], fp32)
            nc.scalar.activation(expB, Bt, Act.Exp)
            sumB = main.tile([C, G], fp32)
            nc.vector.tensor_reduce(
                sumB,
                expB.rearrange("c (m g) -> c g m", m=M),
                mybir.AxisListType.X,
                Alu.add,
            )
            rsumB = main.tile([C, G], fp32)
            nc.vector.reciprocal(rsumB, sumB)
            # scale[c,g] = smPi * (1/sumB);  num = expB * scale (broadcast m)
            scale = main.tile([C, G], fp32)
            nc.vector.tensor_mul(scale, smPi, rsumB)
            num = main.tile([C, M * G], fp32)
            scale_b = bass.AP(
                tensor=scale.tensor,
                offset=scale.offset,
                ap=[scale.ap[0], [0, M], scale.ap[1]],
            )
            nc.vector.tensor_mul(
                num.rearrange("c (m g) -> c m g", m=M),
                expB.rearrange("c (m g) -> c m g", m=M),
                scale_b,
            )
            logN = main.tile([C, M * G], fp32)
            nc.scalar.activation(logN, num, Act.Ln)
            pnl = main.tile([C, M * G], fp32)
            nc.vector.tensor_mul(pnl, num, logN)

            # Z32[m,g] = sum_c num ; ZP32[m,g] = sum_c num*log(num)
            z32 = pp1.tile([M, G], fp32)
            zp32 = pp1.tile([M, G], fp32)
            numv = num.rearrange("c (m g) -> c g m", m=M)
            pnlv = pnl.rearrange("c (m g) -> c g m", m=M)
            for g in range(G):
                nc.tensor.matmul(
                    z32[:, g : g + 1], numv[:, g, :], ones[:, 0:1],
                    start=True, stop=True,
                )
                nc.tensor.matmul(
                    zp32[:, g : g + 1], pnlv[:, g, :], ones[:, 0:1],
                    start=True, stop=True,
                )
            rz32 = main.tile([M, G], fp32)
            nc.vector.reciprocal(rz32, z32)
            L32 = main.tile([M, G], fp32)
            nc.vector.tensor_mul(L32, zp32, rz32)

            # ---- histogram: one-hot compares then slot-reduction tree ----
            OH = main.tile([128, M, J, T], bf16)
            XPf = XP.rearrange("p j t -> p (j t)")
            OHv = OH.rearrange("p m j t -> p m (j t)")
            for m in range(M):
                nc.vector.tensor_scalar(
                    out=OHv[:, m, :],
                    in0=XPf,
                    scalar1=float(m),
                    scalar2=None,
                    op0=Alu.is_equal,
                )

            cur = OH.rearrange("p m j t -> p (m j) t")
            t_sz = T
            while t_sz > 1:
                a = t_sz // 2
                nwid = a + (t_sz % 2)
                nxt = tree.tile([128, M * J, nwid], bf16, tag="tree")
                nc.vector.tensor_add(
                    nxt[:, :, 0:a], cur[:, :, 0:a], cur[:, :, a : 2 * a]
                )
                if t_sz % 2:
                    nc.vector.tensor_copy(
                        nxt[:, :, a : a + 1], cur[:, :, 2 * a : 2 * a + 1]
                    )
                cur = nxt
                t_sz = nwid

            # cur: [128, M*J, 1] bf16 counts; view as [128, j, m] (m stride J)
            cntv = cur.rearrange("p (m j) o -> p j (m o)", j=J)

            # ---- per block: transpose counts, count @ L, negate ----
            OUTS = main.tile([128, J, G], fp32)
            for j in range(J):
                tp = pp.tile([M, 128], bf16, tag="tp")
                nc.tensor.transpose(tp, cntv[:, j, :], identB)
                ct = tree.tile([M, 128], fp32, tag="ct")
                nc.scalar.copy(ct, tp)
                of = pp.tile([128, G], fp32, tag="of")
                nc.tensor.matmul(of, ct, L32, start=True, stop=True)
                nc.scalar.mul(OUTS[:, j, :], of, -1.0)

            nc.sync.dma_start(
                out=out_d.rearrange("(j p) g -> p j g", p=128), in_=OUTS
            )

    nc.compile()
    return nc


def _host_pack(x, batch):
    """Pack node labels into per-graph fixed-stride rows (pad=PAD_LABEL)."""
    import ml_dtypes

    sizes = np.bincount(batch, minlength=N_GRAPHS)
    T = max(32, int(math.ceil(sizes.max() / 32.0)) * 32)
    xp = np.full((N_GRAPHS, T), PAD_LABEL, dtype=np.float32)
    mask = np.arange(T)[None, :] < sizes[:, None]
    # batch is sorted, so row-major True positions match x's node order
    xp[mask] = x.astype(np.float32)
    return xp.astype(ml_dtypes.bfloat16), T


def kernel(x, edge_index, batch, B, Pi):
    import ml_dtypes
    from concourse.bass_utils import run_bass_kernel_spmd

    x = np.asarray(x).astype(np.int64)
    batch = np.asarray(batch).astype(np.int64)
    B = np.asarray(B, dtype=np.float32)
    Pi = np.asarray(Pi, dtype=np.float32)

    xp, T = _host_pack(x, batch)

    nc = _build_nc(T)

    b_flat = np.ascontiguousarray(B.reshape(C, M * G))
    pad_rows = np.full((GPAD - GPC, T), PAD_LABEL, dtype=ml_dtypes.bfloat16)
    in_maps = []
    for i in range(N_CORES):
        shard = np.concatenate([xp[i * GPC : (i + 1) * GPC], pad_rows], axis=0)
        in_maps.append(
            {"xp": np.ascontiguousarray(shard), "B": b_flat, "Pi": Pi}
        )

    res = run_bass_kernel_spmd(
        nc, in_maps, core_ids=list(range(N_CORES)), **_RUN_KWARGS
    )
    kernel.last_results = res
    out = np.concatenate([res.results[i]["out"][:GPC] for i in range(N_CORES)])
    return out[:, None, :].astype(np.float32)


# test harnesses may set _RUN_KWARGS["trace"] = True to collect NTFF timing
_RUN_KWARGS = {}


# revision 29
# speedup vs baseline: 1.5662x; 1.5662x over previous
"""CGMM layer-0 forward on 8 Trainium2 NeuronCores.

Math: per-node likelihood depends only on the node's discrete label
x[n] in [0, 32), so
    lik_node[n, :] = L[x[n], :]         with L a (32, 16) table
    lik_graph[s,:] = sum_m count[s, m] * L[m, :]
where count[s, m] = #{nodes of graph s with label m}.

Sharding: graphs are pre-partitioned across the 8 cores (625 graphs
each, zero cross-core traffic).  Host packs each graph's labels into a
fixed-stride padded row (pad value 64 -> matches no label).  Device:
  - per (graph-block, label): one fused DVE tensor_scalar
    (is_equal + add-reduction via accum_out) producing counts directly
    (bf16 input stream runs in the 4x DVE perf mode)
  - computes L on-device from B, Pi (softmaxes / log via ACT+PE)
  - count @ L via PE transpose + matmul, negate on psum->sbuf copy
"""

import math

import numpy as np

N_NODES = 500_000
N_GRAPHS = 5_000
C = 16
M = 32
G = 16
N_CORES = 8
GPC = N_GRAPHS // N_CORES  # graphs per core = 625
J = 5                      # partition blocks of 128 graphs
GPAD = 128 * J             # padded graphs per core = 640
PAD_LABEL = 64.0


def _build_nc(T_blocks):
    """T_blocks: per-graph-block slot widths (graphs size-sorted descending,
    so later blocks are narrower)."""
    import concourse.bass as bass
    import concourse.bacc as bacc
    import concourse.tile as tile
    import concourse.mybir as mybir
    from concourse.masks import make_identity

    fp32 = mybir.dt.float32
    bf16 = mybir.dt.bfloat16
    Alu = mybir.AluOpType
    Act = mybir.ActivationFunctionType

    nc = bacc.Bacc("TRN2", target_bir_lowering=False, debug=False)

    T0 = T_blocks[0]
    xp_d = nc.dram_tensor("xp", [GPAD, T0], bf16, kind="ExternalInput").ap()
    # params = B (C, M*G) concat Pi (C, G) along the free dim
    par_d = nc.dram_tensor("par", [C, M * G + G], fp32, kind="ExternalInput").ap()
    out_d = nc.dram_tensor("out", [GPAD, G], fp32, kind="ExternalOutput").ap()
    xp_v = xp_d.rearrange("(j p) t -> p j t", p=128)
    out_v = out_d.rearrange("(j p) g -> p j g", p=128)

    with tile.TileContext(nc) as tc:
        with (
            tc.tile_pool(name="main", bufs=1) as main,
            tc.tile_pool(name="xpp", bufs=J) as xpp,
            tc.tile_pool(name="psum", bufs=2, space="PSUM") as pp,
            tc.tile_pool(name="psum1", bufs=1, space="PSUM") as pp1,
        ):
            # ---- histogram inputs first: per-block DMAs so counting can
            # start as soon as block 0 lands ----
            XPj = []
            for j in range(J):
                t_ = xpp.tile([128, T_blocks[j]], bf16, tag=f"xp{j}")
                nc.sync.dma_start(out=t_, in_=xp_v[:, j, 0 : T_blocks[j]])
                XPj.append(t_)

            Par = main.tile([C, M * G + G], fp32)
            nc.scalar.dma_start(out=Par, in_=par_d)
            Bt = Par[:, 0 : M * G]
            Pit = Par[:, M * G : M * G + G]

            # ---- constants ----
            ones = main.tile([C, G], fp32)
            nc.gpsimd.memset(ones, 1.0)
            identF = main.tile([128, 128], fp32)
            make_identity(nc, identF)

            # ---- activations issued early: exps + a dummy Ln to preload
            # the natural-log table set before it's needed mid-stream ----
            expPi = main.tile([C, G], fp32)
            nc.scalar.activation(expPi, Pit, Act.Exp)
            expB = main.tile([C, M * G], fp32)
            nc.scalar.activation(expB, Bt, Act.Exp)
            lnscr = main.tile([C, G], fp32)
            nc.scalar.activation(lnscr, expPi, Act.Ln)

            # ---- histogram: fused compare+reduce, one instr per (j, m),
            # with the (tiny) L-table ops interleaved between blocks ----
            # rotate scratch tiles: a single scratch would chain every
            # instruction on a WAW hazard (engine stalls on the write ack)
            CNT = main.tile([128, J, M], fp32)
            scrs = []
            for i in range(4):
                scr_i = main.tile([128, T0], bf16, tag=f"scr{i}", name=f"scr{i}")
                scrs.append(scr_i)

            # a few (j, m) pairs go to the (mostly idle) scalar engine via
            # count[m] = sum_t relu(1 - |x - m|)  (exact for integer labels)
            ACT_MS = (29, 30, 31)
            ACT_PAIRS = {(j, m) for j in range(1, J) for m in ACT_MS} if _ENABLE_ACT_PAIRS else set()
            ascrs = []
            for i in range(4):
                ascr_i = main.tile([128, T0], bf16, tag=f"ascr{i}", name=f"ascr{i}")
                ascrs.append(ascr_i)
            CNT2 = main.tile([128, J, len(ACT_MS)], fp32)
            one128 = main.tile([128, 1], fp32)
            nc.gpsimd.memset(one128, 1.0)
            negm = {}
            for m in ACT_MS:
                negm_t = main.tile([128, 1], fp32, tag=f"negm{m}", name=f"negm{m}")
                nc.gpsimd.memset(negm_t, float(-m))
                negm[m] = negm_t

            def hist_pair_act(j, m, i):
                tj = T_blocks[j]
                a0, a1 = ascrs[2 * (i % 2)], ascrs[2 * (i % 2) + 1]
                nc.scalar.activation(
                    a0[:, 0:tj], XPj[j], Act.Abs, bias=negm[m], scale=1.0
                )
                nc.scalar.activation(
                    a1[:, 0:tj], a0[:, 0:tj], Act.Relu, bias=one128, scale=-1.0,
                    accum_out=CNT2[:, j, ACT_MS.index(m) : ACT_MS.index(m) + 1],
                )

            def hist_block(j):
                tj = T_blocks[j]
                for m in range(M):
                    if (j, m) in ACT_PAIRS:
                        continue
                    nc.vector.tensor_scalar(
                        out=scrs[m % 4][:, 0:tj],
                        in0=XPj[j],
                        scalar1=float(m),
                        scalar2=0.0,
                        op0=Alu.is_equal,
                        op1=Alu.add,
                        accum_out=CNT[:, j, m : m + 1],
                    )

            hist_block(0)
            for i, (j, m) in enumerate(sorted(ACT_PAIRS)):
                hist_pair_act(j, m, i)

            def merge_block(j):
                if any((j, m) in ACT_PAIRS for m in ACT_MS):
                    nc.vector.tensor_copy(
                        CNT[:, j, ACT_MS[0] : ACT_MS[-1] + 1], CNT2[:, j, :]
                    )

            # ---- L table, part 1: scale[c,g] = smPi[c,g] / sumB[c,g] ----
            # Pi softmax over c (partition axis -> via PE ones-matmuls)
            spi = pp1.tile([1, G], fp32)
            nc.tensor.matmul(spi, ones[:, 0:1], expPi, start=True, stop=True)
            rspi = main.tile([1, G], fp32)
            nc.vector.reciprocal(rspi, spi)
            rspiB = pp1.tile([C, G], fp32)
            nc.tensor.matmul(rspiB, ones[0:1, :], rspi, start=True, stop=True)
            smPi = main.tile([C, G], fp32)
            nc.vector.tensor_mul(smPi, expPi, rspiB)

            sumB = main.tile([C, G], fp32)
            nc.vector.tensor_reduce(
                sumB,
                expB.rearrange("c (m g) -> c g m", m=M),
                mybir.AxisListType.X,
                Alu.add,
            )
            rsumB = main.tile([C, G], fp32)
            nc.vector.reciprocal(rsumB, sumB)
            scale = main.tile([C, G], fp32)
            nc.vector.tensor_mul(scale, smPi, rsumB)
            # eb = expB .* B ; w2 = scale .* ln(scale)
            eb = main.tile([C, M * G], fp32)
            nc.vector.tensor_mul(eb, expB, Bt)
            lnS = main.tile([C, G], fp32)
            nc.scalar.activation(lnS, scale, Act.Ln)
            w2 = main.tile([C, G], fp32)
            nc.vector.tensor_mul(w2, scale, lnS)

            hist_block(1)
            merge_block(1)

            # ---- L table, part 2 (PE): with num = expB*scale and
            # ln(num) = B + ln(scale):
            #   Z32[m,g]  = sum_c expB*scale
            #   ZP32[m,g] = sum_c (expB*B)*scale + sum_c expB*(scale*lnS)
            z32 = pp1.tile([M, G], fp32)
            zp32 = pp1.tile([M, G], fp32)
            expBv = expB.rearrange("c (m g) -> c g m", m=M)
            ebv = eb.rearrange("c (m g) -> c g m", m=M)
            for g in range(G):
                nc.tensor.matmul(
                    z32[:, g : g + 1], expBv[:, g, :], scale[:, g : g + 1],
                    start=True, stop=True,
                )
                nc.tensor.matmul(
                    zp32[:, g : g + 1], ebv[:, g, :], scale[:, g : g + 1],
                    start=True, stop=False,
                )
                nc.tensor.matmul(
                    zp32[:, g : g + 1], expBv[:, g, :], w2[:, g : g + 1],
                    start=False, stop=True,
                )
            rz32 = main.tile([M, G], fp32)
            nc.vector.reciprocal(rz32, z32)
            L32 = main.tile([M, G], fp32)
            nc.vector.tensor_mul(L32, zp32, rz32)

            for j in range(2, J):
                hist_block(j)
                merge_block(j)

            # ---- per block: transpose counts, count @ L, negate, store ----
            OUTS = main.tile([128, J, G], fp32)
            for j in range(J):
                tp = pp.tile([M, 128], fp32, tag="tp")
                nc.tensor.transpose(tp, CNT[:, j, :], identF)
                ct = main.tile([M, 128], fp32, tag=f"ct{j}")
                nc.scalar.copy(ct, tp)
                of = pp.tile([128, G], fp32, tag="of")
                nc.tensor.matmul(of, ct, L32, start=True, stop=True)
                nc.scalar.mul(OUTS[:, j, :], of, -1.0)
                nc.sync.dma_start(out=out_v[:, j, :], in_=OUTS[:, j, :])

    nc.compile()
    return nc


def _host_pack(x, batch):
    """Pack node labels into per-graph fixed-stride rows (pad=PAD_LABEL).

    Within each core's shard of GPC graphs, graphs are sorted by size
    (descending) so later 128-graph blocks need narrower slot widths.
    Returns (per-core padded+sorted xp [N_CORES,GPAD,T0] as bf16,
    T_blocks, per-core orders to un-permute outputs).
    """
    import ml_dtypes

    sizes = np.bincount(batch, minlength=N_GRAPHS)
    T = max(32, int(math.ceil(sizes.max() / 16.0)) * 16)
    xp = np.full((N_GRAPHS, T), PAD_LABEL, dtype=np.float32)
    mask = np.arange(T)[None, :] < sizes[:, None]
    # batch is sorted, so row-major True positions match x's node order
    xp[mask] = x.astype(np.float32)

    shards = []
    orders = []
    block_max = np.zeros(J, dtype=np.int64)
    for i in range(N_CORES):
        s = sizes[i * GPC : (i + 1) * GPC]
        order = np.argsort(-s, kind="stable")
        orders.append(order)
        xs = xp[i * GPC : (i + 1) * GPC][order]
        pad = np.full((GPAD - GPC, T), PAD_LABEL, dtype=np.float32)
        xs = np.concatenate([xs, pad], axis=0)
        shards.append(xs)
        ss = np.concatenate([s[order], np.zeros(GPAD - GPC, dtype=s.dtype)])
        for j in range(J):
            block_max[j] = max(block_max[j], ss[j * 128 : (j + 1) * 128].max())
    T_blocks = [max(16, int(math.ceil(bm / 8.0)) * 8) for bm in block_max]
    T0 = T_blocks[0]
    shards = [np.ascontiguousarray(s[:, :T0].astype(ml_dtypes.bfloat16)) for s in shards]
    return shards, T_blocks, orders


def kernel(x, edge_index, batch, B, Pi):
    import ml_dtypes
    from concourse.bass_utils import run_bass_kernel_spmd

    x = np.asarray(x).astype(np.int64)
    batch = np.asarray(batch).astype(np.int64)
    B = np.asarray(B, dtype=np.float32)
    Pi = np.asarray(Pi, dtype=np.float32)

    shards, T_blocks, orders = _host_pack(x, batch)

    nc = _build_nc(T_blocks)

    par = np.ascontiguousarray(
        np.concatenate([B.reshape(C, M * G), Pi], axis=1)
    )
    in_maps = [{"xp": shards[i], "par": par} for i in range(N_CORES)]

    res = run_bass_kernel_spmd(
        nc, in_maps, core_ids=list(range(N_CORES)), **_RUN_KWARGS
    )
    kernel.last_results = res
    parts = []
    for i in range(N_CORES):
        o_sorted = res.results[i]["out"][:GPC]
        o = np.empty_like(o_sorted)
        o[orders[i]] = o_sorted
        parts.append(o)
    out = np.concatenate(parts)
    return out[:, None, :].astype(np.float32)


# test harnesses may set extra run kwargs (e.g. trace) here
_RUN_KWARGS = {}
_ENABLE_ACT_PAIRS = True
